# revision 8
# baseline (speedup 1.0000x reference)
"""AttentionWithRoPE distributed Trainium2 kernel (8 NeuronCores).

Sharding: pure 8-way tensor parallel over heads (2 heads = 128 hidden cols
per core), both batches on every core (seq concatenated to 4096 cols).
Everything stays transposed ([feature, seq] layouts) so no on-device
transposes are needed anywhere.

The kernel is ScalarE-bound: exp of the 2x[2048,2048] score matrices is
~130us of ACTIVATE at 1 elem/lane/cycle, so everything is arranged to
start exp as early as possible and keep ScalarE saturated:
  phase A: project q/k/v + rope for batch 0 (seq groups 0-3).
  phase C: attention passes for batch 0, with batch 1's q/k projections
           and rope interleaved between passes (PE/DVE have slack under
           the exp-bound passes).
  phase B: v-projection for batch 1 (must precede phase D's ctx matmuls).
  phase D: attention passes for batch 1.
  phase E: AllToAll + keep-warm matmul chain + output projection.

Every dma_start costs ~600ns of SWDGE descriptor-generation time ON THE
ISSUING ENGINE, so DMAs are few and fat: wq|wk|wv are host-packed into one
[128,3072] tensor (1 DMA), cos|sin into one [128,8192] (1 DMA), Wo into
one [128,8192] (1 DMA), biases into [128,2]. The rope half-rotation
(a 32-row partition swap, which DVE cannot do: ops must keep operand
start-partitions equal) is done ON THE TENSOR ENGINE as a matmul with a
host-supplied 0/1 permutation matrix; the sin-multiply then reads the
swapped copy straight out of PSUM. x streams on the Sync (chunks 0-3) and
GpSimd (chunks 4-7) queues; softmax-reciprocal hops and a2a_in stores go
on GpSimd's queue. ScalarE issues only the 4 const loads at t=0.

Attention details:
  - scores^T = kT.T @ qT per (head, batch) in [ks, qs] layout as K=64
    row-tiled matmul pairs: head0 streams through PE rows 0-63 while head1
    streams through rows 64-127 concurrently (auto tile_position (0,0) /
    (64,0) from the operands' base partitions; verified dStart ~3ns).
  - exp on ScalarE in [128,1536]-wide ops over 3-bank psum score tiles
    ((N+352)/1.2ns each, so wider = less per-op overhead); no max
    subtraction needed: scores are ~N(0,1) here.
  - ctx^T via M=65 matmuls with a ones-column appended to V (the 65th
    column gives the softmax denominator for free). V lives in a single 3D
    tile [128, 64, 65] (slot = key-block*2+head); ones columns initialized
    by ONE strided memset.
  - normalization: reciprocal of the rowsum row in place (partition 64),
    one [1,512] hop DMA to partition 0, GpSimd partition-broadcast, one
    DVE multiply (fuses psum->sbuf copy + cast to bf16).
  - PSUM: 2x 3-bank slots (q/k proj + 1536-wide scores) + 2x 1-bank slots
    (v/rope-swap psums, then the two ctx accumulators) = 8 banks exactly.
  - AllToAll (bf16, all 8 cores) exchanges 512-row blocks of ctx^T;
    received slabs are exactly the o-chunks the output projection consumes.
  - output projection with full Wo produces out^T [1024, 512] for this
    core's 512 global rows; host transposes back (free).
Bias folds (host side): v-bias folds into the output bias exactly (softmax
rows sum to 1); q is pre-scaled by 1/sqrt(64) inside its bias-copy.
Compute dtype bf16 (fp32 PSUM accumulation).
"""

import numpy as np

HID = 1024
S = 2048
SB = 2 * S       # both batches, seq-concatenated
NHEAD = 16
D = 64
HPC = 2          # heads per core
OSL = 128        # hidden slice per core (HPC * D)
RB = 512         # global row block per core after AllToAll
NC = 8
ROPE_BASE = 10000.0

_cached = None
_last_in_maps = None

N_DUMMY = 150    # keep-PE-warm matmuls spanning the AllToAll wait


def _build_nc():
    import concourse.bacc as bacc
    import concourse.mybir as mybir
    from concourse import tile

    f32 = mybir.dt.float32
    bf16 = mybir.dt.bfloat16
    AF = mybir.ActivationFunctionType

    nc = bacc.Bacc(None, target_bir_lowering=False)

    xT = nc.declare_dram_parameter("xT", [HID, SB], bf16, isOutput=False)
    wqkvd = nc.declare_dram_parameter("wqkv", [128, 3 * HID], bf16,
                                      isOutput=False)
    wod = nc.declare_dram_parameter("woL", [128, 8 * HID], bf16,
                                    isOutput=False)
    bqkd = nc.declare_dram_parameter("bqk", [128, 2], f32, isOutput=False)
    bod = nc.declare_dram_parameter("bo2", [128, 8], f32, isOutput=False)
    csd = nc.declare_dram_parameter("cs", [128, 2 * SB], bf16,
                                    isOutput=False)
    permd = nc.declare_dram_parameter("perm", [128, 128], bf16,
                                      isOutput=False)
    out_ext = nc.declare_dram_parameter("out", [HID, RB], bf16, isOutput=True)

    a2a_in = nc.dram_tensor("a2a_in", [NC, OSL, RB], bf16)
    a2a_out = nc.dram_tensor("a2a_out", [NC, OSL, RB], bf16)

    NHC = HID // 128  # 8 hidden chunks

    with tile.TileContext(nc) as tc:
        with (
            tc.tile_pool(name="persist", bufs=1) as pp,
            tc.tile_pool(name="xs", bufs=16) as xp,
            tc.tile_pool(name="work", bufs=2) as wp,
            tc.tile_pool(name="exp", bufs=2) as ep,
        ):
            # ---------- consts: 4 fat DMAs on ScalarE's queue at t=0 ------
            wqkv = pp.tile([128, 3 * HID], bf16, tag="wqkv", name="wqkv")
            nc.scalar.dma_start(out=wqkv[:, :], in_=wqkvd[:, :])
            bqk = pp.tile([128, 2], f32, tag="bqk", name="bqk")
            nc.scalar.dma_start(out=bqk[:, :], in_=bqkd[:, :])
            perm = pp.tile([128, 128], bf16, tag="perm", name="perm")
            nc.scalar.dma_start(out=perm[:, :], in_=permd[:, :])
            cs = pp.tile([128, 2 * SB], bf16, tag="cs", name="cs")
            nc.scalar.dma_start(out=cs[:, :], in_=csd[:, :])
            bo_sb = pp.tile([128, 8], f32, tag="bo", name="bo")
            nc.scalar.dma_start(out=bo_sb[:, :], in_=bod[:, :])

            def wsl(t, c):      # lhsT slice for projection t in (q,k,v)
                return wqkv[:, 1024 * t + 128 * c:1024 * t + 128 * (c + 1)]

            # PSUM pools: "spsbig" = 2x 3-bank slots, "acc" = 2x 1-bank.
            _cmA = tc.tile_pool(name="psA", bufs=2, space="PSUM")
            _cmB = tc.tile_pool(name="psB", bufs=2, space="PSUM")
            psA = _cmA.__enter__()
            psB = _cmB.__enter__()

            qsb = wp.tile([128, SB], bf16, tag="qsb", bufs=1)
            ksb = wp.tile([128, SB], bf16, tag="ksb", bufs=1)
            # Packed rope outputs: head0 d0-63 on partitions 0-63, head1 on
            # 64-127, consumed by K=64 row-tiled score matmul pairs.
            qr = pp.tile([128, SB], bf16, tag="qr", name="qr")
            kr = pp.tile([128, SB], bf16, tag="kr", name="kr")
            vt = pp.tile([128, 2 * SB // 128, D + 1], bf16, tag="vt",
                         name="vt")
            nc.gpsimd.memset(vt[:, :, D:D + 1], 1.0)
            ctxh = [pp.tile([64, SB], bf16, tag=f"ctx{h}", name=f"ctx{h}")
                    for h in range(HPC)]

            # ---------- emission helpers ----------
            def load_x(sp):
                xbt = []
                for c in range(NHC):
                    xb = xp.tile([128, 1024], bf16, tag="xb", bufs=16)
                    eng = nc.sync if c < 4 else nc.gpsimd
                    eng.dma_start(
                        out=xb[:, :],
                        in_=xT[128 * c:128 * (c + 1),
                               1024 * sp:1024 * (sp + 1)])
                    xbt.append(xb)
                return xbt

            def qk_sg(sg, xbt):
                half = sg % 2
                xsl = slice(512 * half, 512 * (half + 1))
                ps = psA.tile([128, 1024], f32, tag="spsbig",
                              padded_shape=[128, 1536], name=f"qkps{sg}")
                for c in range(NHC):
                    nc.tensor.matmul(
                        ps[:, 0:512], lhsT=wsl(0, c), rhs=xbt[c][:, xsl],
                        start=(c == 0), stop=(c == NHC - 1))
                for c in range(NHC):
                    nc.tensor.matmul(
                        ps[:, 512:1024], lhsT=wsl(1, c), rhs=xbt[c][:, xsl],
                        start=(c == 0), stop=(c == NHC - 1))
                nc.vector.tensor_scalar(
                    qsb[:, 512 * sg:512 * (sg + 1)],
                    ps[:, 0:512], 0.125, bqk[:, 0:1],
                    mybir.AluOpType.mult, mybir.AluOpType.add)
                nc.vector.tensor_scalar(
                    ksb[:, 512 * sg:512 * (sg + 1)],
                    ps[:, 512:1024], 1.0, bqk[:, 1:2],
                    mybir.AluOpType.mult, mybir.AluOpType.add)

            def rope_sg(sg):
                # rope on the 512-col block of both q and k: the 32-row
                # half-rotation is a PE matmul against the permutation
                # matrix; sin-multiply reads the swap from PSUM.
                sl = slice(512 * sg, 512 * (sg + 1))
                for src, dst in ((qsb, qr), (ksb, kr)):
                    swp = psB.tile([128, 512], f32, tag="acc")
                    nc.tensor.matmul(swp[:, :], lhsT=perm[:, :],
                                     rhs=src[:, sl], start=True, stop=True)
                    t1 = wp.tile([128, 512], f32, tag="ropet1")
                    nc.vector.tensor_mul(t1[:, :], src[:, sl], cs[:, sl])
                    t2 = wp.tile([128, 512], f32, tag="ropet2")
                    nc.vector.tensor_mul(t2[:, :], swp[:, :],
                                         cs[:, SB + 512 * sg:SB + 512 * (sg + 1)])
                    nc.vector.tensor_add(dst[:, sl], t1[:, :], t2[:, :])

            def v_sg(sg, xbt):
                half = sg % 2
                for st4 in range(4):
                    st = 4 * sg + st4
                    ps = psB.tile([128, OSL], f32, tag="acc",
                                  padded_shape=[128, 512])
                    x0 = 512 * half + 128 * st4
                    for c in range(NHC):
                        nc.tensor.matmul(
                            ps[:, :],
                            lhsT=xbt[c][:, x0:x0 + 128],
                            rhs=wsl(2, c),
                            start=(c == 0), stop=(c == NHC - 1))
                    for h in range(HPC):
                        nc.vector.tensor_copy(
                            vt[:, 2 * st + h, 0:D],
                            ps[:, 64 * h:64 * (h + 1)])

            def attn_pass(b, qs):
                q0 = S * b + 512 * qs
                cpsA = psB.tile([128, 512], f32, tag="acc")
                cpsB = psB.tile([128, 512], f32, tag="acc")

                def exp_ctx(sps, us):
                    W = 512 * len(us)
                    et = ep.tile([128, 1536], bf16, tag="expT", bufs=6)
                    nc.scalar.activation(et[:, 0:W], sps[:, 0:W], AF.Exp)
                    for j, u in enumerate(us):
                        ks, h = divmod(u, 2)
                        kb = 16 * b + ks
                        nc.tensor.matmul(
                            (cpsA if h == 0 else cpsB)[0:D + 1, :],
                            lhsT=vt[:, 2 * kb + h, :],
                            rhs=et[:, 512 * j:512 * (j + 1)],
                            start=(ks == 0), stop=(ks == 15))

                pend = None
                for T in range(11):
                    us = list(range(3 * T, min(3 * T + 3, 32)))
                    W = 512 * len(us)
                    sps = psA.tile([128, W], f32, tag="spsbig",
                                   padded_shape=[128, 1536])
                    for j, u in enumerate(us):
                        ks, h = divmod(u, 2)
                        k0 = S * b + 128 * ks
                        nc.tensor.matmul(
                            sps[:, 512 * j:512 * (j + 1)],
                            lhsT=kr[64 * h:64 * (h + 1), k0:k0 + 128],
                            rhs=qr[64 * h:64 * (h + 1), q0:q0 + 512],
                            start=True, stop=True)
                    if pend is not None:
                        exp_ctx(*pend)
                    pend = (sps, us)
                exp_ctx(*pend)

                for h, cps in ((0, cpsA), (1, cpsB)):
                    # rowsum lives on psum partition 64; reciprocal in
                    # place, hop to partition 0, broadcast, normalize.
                    rec65 = ep.tile([65, 512], f32, tag="rec65")
                    nc.vector.reciprocal(rec65[64:65, :], cps[64:65, :])
                    rec0 = ep.tile([1, 512], f32, tag="rec0")
                    nc.gpsimd.dma_start(out=rec0[:, :],
                                        in_=rec65[64:65, :])
                    rb = ep.tile([64, 512], f32, tag="recb")
                    nc.gpsimd.partition_broadcast(rb[:, :], rec0[:, :])
                    nc.vector.tensor_mul(
                        ctxh[h][:, q0:q0 + 512], cps[0:64, :], rb[:, :])
                    # this (b,qs) round is destination-slab 4b+qs: ship it
                    # to the a2a buffer as soon as it is normalized
                    nc.gpsimd.dma_start(
                        out=a2a_in[4 * b + qs, 64 * h:64 * (h + 1), :],
                        in_=ctxh[h][:, q0:q0 + 512])

            # ---------- phase A: batch-0 projections + rope ----------
            xb0 = load_x(0)
            xb1 = load_x(1)
            for sg in range(4):
                xbt = xb0 if sg < 2 else xb1
                qk_sg(sg, xbt)
                rope_sg(sg)
                v_sg(sg, xbt)

            xb2 = load_x(2)
            xb3 = load_x(3)
            # wo is needed first by the keep-warm chain right after the last
            # pass; stream it on the (idle) Sync queue during attention.
            wo_sb = pp.tile([128, 8 * HID], bf16, tag="woL", name="wo_sb")
            nc.sync.dma_start(out=wo_sb[:, :], in_=wod[:, :])

            # ---------- phase C: batch-0 attention (+ batch-1 q/k + rope) --
            for i in range(4):
                attn_pass(0, i)
                sg = 4 + i
                qk_sg(sg, xb2 if i < 2 else xb3)
                rope_sg(sg)

            # ---------- phase B: batch-1 v-projection ----------
            for sg in range(4, 8):
                v_sg(sg, xb2 if sg < 6 else xb3)

            # ---------- phase D: batch-1 attention ----------
            for qs in range(4):
                attn_pass(1, qs)

            # ---------- phase E: AllToAll + output projection ----------
            nc.gpsimd.collective_compute(
                "AllToAll", mybir.AluOpType.bypass,
                replica_groups=[list(range(NC))],
                ins=[a2a_in.ap().opt()],
                outs=[a2a_out.ap().opt()])

            _cmB.__exit__(None, None, None)
            _cmA.__exit__(None, None, None)
            _cmO = tc.tile_pool(name="psO", bufs=1, space="PSUM")
            psO = _cmO.__enter__()

            # Keep the PE array (HAM) warm across the AllToAll wait: a chain
            # of full-array matmuls anchored on the last ctx tile (via the
            # dumsrc copy) so they cannot run before attention finishes.
            # Result is consumed by a dead-store DMA so DCE keeps the chain.
            dumsrc = pp.tile([128, 512], bf16, tag="dumsrc")
            nc.gpsimd.memset(dumsrc[:, :], 0.0)
            nc.vector.tensor_copy(
                dumsrc[0:64, :], ctxh[1][:, SB - 512:SB])
            dum = psO.tile([128, 512], f32, tag="dum", bufs=1)
            for i in range(N_DUMMY):
                nc.tensor.matmul(
                    dum[:, :], lhsT=wo_sb[:, 0:128], rhs=dumsrc[:, :],
                    start=True, stop=True)
            dumr = ep.tile([128, 512], f32, tag="dumr")
            nc.vector.tensor_copy(dumr[:, :], dum[:, :])
            dead = nc.dram_tensor("dead", [128, 512], f32)
            nc.sync.dma_start(out=dead[:, :], in_=dumr[:, :])
            # Load all 8 received o-chunks (split across two queues), then
            # run the accumulation ot-outer so each out-tile finishes early
            # and its bias-add + store overlap the remaining matmuls.
            cxs = []
            for c in range(NHC):
                cx = pp.tile([128, RB], bf16, tag=f"cxb{c}", name=f"cxb{c}")
                eng = nc.sync if c % 2 == 0 else nc.gpsimd
                eng.dma_start(out=cx[:, :], in_=a2a_out[c, :, :])
                cxs.append(cx)
            for ot in range(8):
                ops = psO.tile([128, 512], f32, tag="ops", bufs=4)
                for c in range(NHC):
                    nc.tensor.matmul(
                        ops[:, :],
                        lhsT=wo_sb[:, 1024 * c + 128 * ot:
                                   1024 * c + 128 * (ot + 1)],
                        rhs=cxs[c][:, :],
                        start=(c == 0), stop=(c == NHC - 1))
                osb = ep.tile([128, RB], bf16, tag="osb", bufs=3)
                nc.scalar.activation(
                    osb[:, :], ops[:, :], AF.Identity,
                    bias=bo_sb[:, ot:ot + 1], scale=1.0)
                eng = nc.sync if ot % 2 == 0 else nc.gpsimd
                eng.dma_start(
                    out=out_ext[128 * ot:128 * (ot + 1), :], in_=osb[:, :])
            _cmO.__exit__(None, None, None)

    nc.finalize()
    return nc


def _host_tables():
    inv = 1.0 / (ROPE_BASE ** (np.arange(0, D, 2, dtype=np.float64) / D))
    pos = np.arange(S, dtype=np.float64)
    freqs = np.outer(pos, inv)                      # [S, 32]
    emb = np.concatenate([freqs, freqs], axis=-1)   # [S, 64]
    cosT = np.cos(emb).T.astype(np.float32)         # [64, S]
    sinT = np.sin(emb).T.astype(np.float32)
    sinS = np.concatenate([-sinT[:32], sinT[32:]], axis=0)
    cos2 = np.ascontiguousarray(np.tile(cosT, (2, 2)))   # [128, 2S]
    sin2 = np.ascontiguousarray(np.tile(sinS, (2, 2)))
    return cos2, sin2


def _pack_wqkv(Wq, Wk, Wv, sl, bf):
    # [128, 3*1024]: col t*1024 + c*128 + j holds W_t.T[128c+p, j] for
    # partition p (pre-transposed row chunks of each projection).
    out = np.empty((128, 3 * 1024), dtype=np.float32)
    for t, W in enumerate((Wq, Wk, Wv)):
        wt = W[sl, :].T.reshape(8, 128, 128)          # [c, p, j]
        out[:, 1024 * t:1024 * (t + 1)] = (
            wt.transpose(1, 0, 2).reshape(128, 1024))
    return np.ascontiguousarray(out).astype(bf)


def kernel(**inputs):
    import ml_dtypes
    from concourse.bass_utils import run_bass_kernel_spmd

    global _cached, _last_in_maps
    if _cached is None:
        _cached = _build_nc()
    nc = _cached

    bf = ml_dtypes.bfloat16
    hs = np.asarray(inputs["hidden_states"], dtype=np.float32)
    Wq = np.asarray(inputs["Wq"], dtype=np.float32)
    bq = np.asarray(inputs["bq"], dtype=np.float32)
    Wk = np.asarray(inputs["Wk"], dtype=np.float32)
    bk = np.asarray(inputs["bk"], dtype=np.float32)
    Wv = np.asarray(inputs["Wv"], dtype=np.float32)
    bv = np.asarray(inputs["bv"], dtype=np.float32)
    Wo = np.asarray(inputs["Wo"], dtype=np.float32)
    bo = np.asarray(inputs["bo"], dtype=np.float32)

    cos2, sin2 = _host_tables()
    cs = np.ascontiguousarray(
        np.concatenate([cos2, sin2], axis=1)).astype(bf)   # [128, 2SB]
    bo2 = bo + bv @ Wo.T                                 # fold v-bias exactly
    bo2m = np.ascontiguousarray(bo2.reshape(8, 128).T)   # [128, 8]
    xTfull = np.ascontiguousarray(
        np.concatenate([hs[0].T, hs[1].T], axis=1)).astype(bf)  # [1024, 4096]
    # woL[p, 1024c + m] = Wo.T[128c+p, m]
    woL = np.ascontiguousarray(
        Wo.T.reshape(8, 128, 1024).transpose(1, 0, 2).reshape(128, 8192)
    ).astype(bf)
    # 32-row half-rotation permutation: dest row m reads source row perm(m)
    pidx = np.arange(128)
    pm = np.where(pidx % 64 < 32, pidx + 32, pidx - 32)
    permM = np.zeros((128, 128), dtype=np.float32)
    permM[pm, pidx] = 1.0                                # [k, m]: k==perm(m)
    permM = permM.astype(bf)

    in_maps = []
    for c in range(NC):
        sl = slice(OSL * c, OSL * (c + 1))
        bqk = np.stack([bq[sl] * 0.125, bk[sl]], axis=1)  # [128, 2]
        in_maps.append({
            "xT": xTfull,
            "wqkv": _pack_wqkv(Wq, Wk, Wv, sl, bf),
            "woL": woL,
            "bqk": np.ascontiguousarray(bqk.astype(np.float32)),
            "bo2": bo2m,
            "cs": cs,
            "perm": permM,
        })

    _last_in_maps = in_maps
    res = run_bass_kernel_spmd(nc, in_maps, core_ids=list(range(NC)))
    out = np.empty((2, S, HID), dtype=np.float32)
    for c in range(NC):
        b, g = divmod(c, 4)
        out[b, RB * g:RB * (g + 1), :] = res.results[c]["out"].T.astype(np.float32)
    return out


# revision 10
# speedup vs baseline: 1.1847x; 1.1847x over previous
"""AttentionWithRoPE distributed Trainium2 kernel (8 NeuronCores).

Sharding: pure 8-way tensor parallel over heads (2 heads = 128 hidden cols
per core), both batches on every core (seq concatenated to 4096 cols).
Everything stays transposed ([feature, seq] layouts) so no on-device
transposes are needed anywhere.

The kernel is ScalarE-bound: exp of the 2x[2048,2048] score matrices is
~130us of ACTIVATE at 1 elem/lane/cycle, so everything is arranged to
start exp as early as possible and keep ScalarE saturated:
  phase A: project q/k/v + rope for batch 0 (seq groups 0-3).
  phase C: attention passes for batch 0, with batch 1's q/k projections
           and rope interleaved between passes (PE/DVE have slack under
           the exp-bound passes).
  phase B: v-projection for batch 1 (must precede phase D's ctx matmuls).
  phase D: attention passes for batch 1.
  phase E: AllToAll + keep-warm matmul chain + output projection.

Every dma_start costs ~600ns of SWDGE descriptor-generation time ON THE
ISSUING ENGINE, so DMAs are few and fat: wq|wk|wv are host-packed into one
[128,3072] tensor (1 DMA), cos|sin into one [128,8192] (1 DMA), Wo into
one [128,8192] (1 DMA), biases into [128,2]. The rope half-rotation
(a 32-row partition swap, which DVE cannot do: ops must keep operand
start-partitions equal) is done ON THE TENSOR ENGINE as a matmul with a
host-supplied 0/1 permutation matrix; the sin-multiply then reads the
swapped copy straight out of PSUM. x streams on the Sync (chunks 0-3) and
GpSimd (chunks 4-7) queues; softmax-reciprocal hops and a2a_in stores go
on GpSimd's queue. ScalarE issues only the 4 const loads at t=0.

Attention details:
  - scores^T = kT.T @ qT per (head, batch) in [ks, qs] layout as K=64
    row-tiled matmul pairs: head0 streams through PE rows 0-63 while head1
    streams through rows 64-127 concurrently (auto tile_position (0,0) /
    (64,0) from the operands' base partitions; verified dStart ~3ns).
  - exp on ScalarE in [128,1536]-wide ops over 3-bank psum score tiles
    ((N+352)/1.2ns each, so wider = less per-op overhead); no max
    subtraction needed: scores are ~N(0,1) here.
  - ctx^T via M=65 matmuls with a ones-column appended to V (the 65th
    column gives the softmax denominator for free). V lives in a single 3D
    tile [128, 64, 65] (slot = key-block*2+head); ones columns initialized
    by ONE strided memset.
  - normalization: reciprocal of the rowsum row in place (partition 64),
    one [1,512] hop DMA to partition 0, GpSimd partition-broadcast, one
    DVE multiply (fuses psum->sbuf copy + cast to bf16).
  - PSUM: 2x 3-bank slots (q/k proj + 1536-wide scores) + 2x 1-bank slots
    (v/rope-swap psums, then the two ctx accumulators) = 8 banks exactly.
  - AllToAll (bf16, all 8 cores) exchanges 512-row blocks of ctx^T;
    received slabs are exactly the o-chunks the output projection consumes.
  - output projection with full Wo produces out^T [1024, 512] for this
    core's 512 global rows; host transposes back (free).
Bias folds (host side): v-bias folds into the output bias exactly (softmax
rows sum to 1); q is pre-scaled by 1/sqrt(64) inside its bias-copy.
Compute dtype bf16 (fp32 PSUM accumulation).
"""

import numpy as np

HID = 1024
S = 2048
SB = 2 * S       # both batches, seq-concatenated
NHEAD = 16
D = 64
HPC = 2          # heads per core
OSL = 128        # hidden slice per core (HPC * D)
RB = 512         # global row block per core after AllToAll
NC = 8
ROPE_BASE = 10000.0

_cached = None
_last_in_maps = None

N_DUMMY = 150    # keep-PE-warm matmuls spanning the AllToAll wait


def _build_nc():
    import concourse.bacc as bacc
    import concourse.mybir as mybir
    from concourse import tile

    f32 = mybir.dt.float32
    bf16 = mybir.dt.bfloat16
    AF = mybir.ActivationFunctionType

    nc = bacc.Bacc(None, target_bir_lowering=False)

    xT = nc.declare_dram_parameter("xT", [HID, SB], bf16, isOutput=False)
    wqkvd = nc.declare_dram_parameter("wqkv", [128, 3 * HID], bf16,
                                      isOutput=False)
    wod = nc.declare_dram_parameter("woL", [128, 8 * HID], bf16,
                                    isOutput=False)
    bqkd = nc.declare_dram_parameter("bqk", [128, 2], f32, isOutput=False)
    bod = nc.declare_dram_parameter("bo2", [128, 8], f32, isOutput=False)
    csd = nc.declare_dram_parameter("cs", [128, 2 * SB], bf16,
                                    isOutput=False)
    permd = nc.declare_dram_parameter("perm", [128, 128], bf16,
                                      isOutput=False)
    out_ext = nc.declare_dram_parameter("out", [HID, RB], bf16, isOutput=True)

    a2a_in = nc.dram_tensor("a2a_in", [NC, OSL, RB], bf16)
    a2a_out = nc.dram_tensor("a2a_out", [NC, OSL, RB], bf16)

    NHC = HID // 128  # 8 hidden chunks

    with tile.TileContext(nc) as tc:
        with (
            tc.tile_pool(name="persist", bufs=1) as pp,
            tc.tile_pool(name="xs", bufs=16) as xp,
            tc.tile_pool(name="work", bufs=2) as wp,
            tc.tile_pool(name="exp", bufs=2) as ep,
        ):
            # ---------- consts: 4 fat DMAs on ScalarE's queue at t=0 ------
            wqkv = pp.tile([128, 3 * HID], bf16, tag="wqkv", name="wqkv")
            nc.scalar.dma_start(out=wqkv[:, :], in_=wqkvd[:, :])
            bqk = pp.tile([128, 2], f32, tag="bqk", name="bqk")
            nc.scalar.dma_start(out=bqk[:, :], in_=bqkd[:, :])
            perm = pp.tile([128, 128], bf16, tag="perm", name="perm")
            nc.scalar.dma_start(out=perm[:, :], in_=permd[:, :])
            cs = pp.tile([128, 2 * SB], bf16, tag="cs", name="cs")
            for half in range(2):      # cos|sin for batch0 first, batch1 next
                for part in range(2):
                    lo = SB * part + S * half
                    nc.scalar.dma_start(out=cs[:, lo:lo + S],
                                        in_=csd[:, lo:lo + S])
            bo_sb = pp.tile([128, 8], f32, tag="bo", name="bo")
            nc.scalar.dma_start(out=bo_sb[:, :], in_=bod[:, :])

            def wsl(t, c):      # lhsT slice for projection t in (q,k,v)
                return wqkv[:, 1024 * t + 128 * c:1024 * t + 128 * (c + 1)]

            # PSUM pools: "spsbig" = 2x 3-bank slots, "acc" = 2x 1-bank.
            _cmA = tc.tile_pool(name="psA", bufs=2, space="PSUM")
            _cmB = tc.tile_pool(name="psB", bufs=2, space="PSUM")
            psA = _cmA.__enter__()
            psB = _cmB.__enter__()

            qsb = wp.tile([128, SB], bf16, tag="qsb", bufs=1)
            ksb = wp.tile([128, SB], bf16, tag="ksb", bufs=1)
            # Packed rope outputs: head0 d0-63 on partitions 0-63, head1 on
            # 64-127, consumed by K=64 row-tiled score matmul pairs.
            qr = pp.tile([128, SB], bf16, tag="qr", name="qr")
            kr = pp.tile([128, SB], bf16, tag="kr", name="kr")
            vt = pp.tile([128, 2 * SB // 128, D + 1], bf16, tag="vt",
                         name="vt")
            nc.gpsimd.memset(vt[:, :, D:D + 1], 1.0)
            ctxh = [pp.tile([64, SB], bf16, tag=f"ctx{h}", name=f"ctx{h}")
                    for h in range(HPC)]

            # ---------- emission helpers ----------
            def load_x(sp):
                xbt = []
                for c in range(NHC):
                    xb = xp.tile([128, 1024], bf16, tag="xb", bufs=16)
                    eng = nc.sync if c < 4 else nc.gpsimd
                    eng.dma_start(
                        out=xb[:, :],
                        in_=xT[128 * c:128 * (c + 1),
                               1024 * sp:1024 * (sp + 1)])
                    xbt.append(xb)
                return xbt

            def qk_sg(sg, xbt):
                half = sg % 2
                xsl = slice(512 * half, 512 * (half + 1))
                ps = psA.tile([128, 1024], f32, tag="spsbig",
                              padded_shape=[128, 1536], name=f"qkps{sg}")
                for c in range(NHC):
                    nc.tensor.matmul(
                        ps[:, 0:512], lhsT=wsl(0, c), rhs=xbt[c][:, xsl],
                        start=(c == 0), stop=(c == NHC - 1))
                for c in range(NHC):
                    nc.tensor.matmul(
                        ps[:, 512:1024], lhsT=wsl(1, c), rhs=xbt[c][:, xsl],
                        start=(c == 0), stop=(c == NHC - 1))
                nc.vector.tensor_scalar(
                    qsb[:, 512 * sg:512 * (sg + 1)],
                    ps[:, 0:512], 0.125, bqk[:, 0:1],
                    mybir.AluOpType.mult, mybir.AluOpType.add)
                nc.vector.tensor_scalar(
                    ksb[:, 512 * sg:512 * (sg + 1)],
                    ps[:, 512:1024], 1.0, bqk[:, 1:2],
                    mybir.AluOpType.mult, mybir.AluOpType.add)

            def rope_sg(sg):
                # rope on the 512-col block of both q and k: the 32-row
                # half-rotation is a PE matmul against the permutation
                # matrix; sin-multiply reads the swap from PSUM.
                sl = slice(512 * sg, 512 * (sg + 1))
                for src, dst in ((qsb, qr), (ksb, kr)):
                    swp = psB.tile([128, 512], f32, tag="acc")
                    nc.tensor.matmul(swp[:, :], lhsT=perm[:, :],
                                     rhs=src[:, sl], start=True, stop=True)
                    t1 = wp.tile([128, 512], f32, tag="ropet1")
                    nc.vector.tensor_mul(t1[:, :], src[:, sl], cs[:, sl])
                    t2 = wp.tile([128, 512], f32, tag="ropet2")
                    nc.vector.tensor_mul(t2[:, :], swp[:, :],
                                         cs[:, SB + 512 * sg:SB + 512 * (sg + 1)])
                    nc.vector.tensor_add(dst[:, sl], t1[:, :], t2[:, :])

            def v_sg(sg, xbt):
                half = sg % 2
                for st4 in range(4):
                    st = 4 * sg + st4
                    ps = psB.tile([128, OSL], f32, tag="acc",
                                  padded_shape=[128, 512])
                    x0 = 512 * half + 128 * st4
                    for c in range(NHC):
                        nc.tensor.matmul(
                            ps[:, :],
                            lhsT=xbt[c][:, x0:x0 + 128],
                            rhs=wsl(2, c),
                            start=(c == 0), stop=(c == NHC - 1))
                    for h in range(HPC):
                        nc.vector.tensor_copy(
                            vt[:, 2 * st + h, 0:D],
                            ps[:, 64 * h:64 * (h + 1)])

            def attn_pass(b, qs):
                q0 = S * b + 512 * qs
                cpsA = psB.tile([128, 512], f32, tag="acc")
                cpsB = psB.tile([128, 512], f32, tag="acc")

                def exp_ctx(sps, us):
                    W = 512 * len(us)
                    et = ep.tile([128, 1536], bf16, tag="expT", bufs=6)
                    nc.scalar.activation(et[:, 0:W], sps[:, 0:W], AF.Exp)
                    for j, u in enumerate(us):
                        ks, h = divmod(u, 2)
                        kb = 16 * b + ks
                        nc.tensor.matmul(
                            (cpsA if h == 0 else cpsB)[0:D + 1, :],
                            lhsT=vt[:, 2 * kb + h, :],
                            rhs=et[:, 512 * j:512 * (j + 1)],
                            start=(ks == 0), stop=(ks == 15))

                pend = None
                for T in range(11):
                    us = list(range(3 * T, min(3 * T + 3, 32)))
                    W = 512 * len(us)
                    sps = psA.tile([128, W], f32, tag="spsbig",
                                   padded_shape=[128, 1536])
                    for j, u in enumerate(us):
                        ks, h = divmod(u, 2)
                        k0 = S * b + 128 * ks
                        nc.tensor.matmul(
                            sps[:, 512 * j:512 * (j + 1)],
                            lhsT=kr[64 * h:64 * (h + 1), k0:k0 + 128],
                            rhs=qr[64 * h:64 * (h + 1), q0:q0 + 512],
                            start=True, stop=True)
                    if pend is not None:
                        exp_ctx(*pend)
                    pend = (sps, us)
                exp_ctx(*pend)

                for h, cps in ((0, cpsA), (1, cpsB)):
                    # rowsum lives on psum partition 64; hop it to sbuf,
                    # DMA-reshape to [128,4] so reciprocal runs 128 lanes
                    # wide (single-partition reciprocal is ~8 cyc/elem),
                    # reshape back, broadcast, normalize. Hop DMAs ride the
                    # Sync queue (idle during attention).
                    rs65 = ep.tile([65, 512], f32, tag="rec65")
                    nc.vector.tensor_copy(rs65[64:65, :], cps[64:65, :])
                    rsP = ep.tile([128, 4], f32, tag="rsP")
                    nc.sync.dma_start(out=rsP[:, :], in_=rs65[64:65, :])
                    rPr = ep.tile([128, 4], f32, tag="rPr")
                    nc.vector.reciprocal(rPr[:, :], rsP[:, :])
                    rec0 = ep.tile([1, 512], f32, tag="rec0")
                    nc.sync.dma_start(out=rec0[:, :], in_=rPr[:, :])
                    rb = ep.tile([64, 512], f32, tag="recb")
                    nc.gpsimd.partition_broadcast(rb[:, :], rec0[:, :])
                    nc.vector.tensor_mul(
                        ctxh[h][:, q0:q0 + 512], cps[0:64, :], rb[:, :])
                    # this (b,qs) round is destination-slab 4b+qs: ship it
                    # to the a2a buffer as soon as it is normalized
                    nc.gpsimd.dma_start(
                        out=a2a_in[4 * b + qs, 64 * h:64 * (h + 1), :],
                        in_=ctxh[h][:, q0:q0 + 512])

            # ---------- phase A: batch-0 projections + rope ----------
            xb0 = load_x(0)
            xb1 = load_x(1)
            for sg in range(4):
                xbt = xb0 if sg < 2 else xb1
                qk_sg(sg, xbt)
                rope_sg(sg)
                v_sg(sg, xbt)

            xb2 = load_x(2)
            xb3 = load_x(3)
            # wo is needed first by the keep-warm chain right after the last
            # pass; stream it on the (idle) Sync queue during attention.
            wo_sb = pp.tile([128, 8 * HID], bf16, tag="woL", name="wo_sb")
            nc.sync.dma_start(out=wo_sb[:, :], in_=wod[:, :])

            # ---------- phase C: batch-0 attention (+ batch-1 q/k + rope) --
            for i in range(4):
                attn_pass(0, i)
                sg = 4 + i
                qk_sg(sg, xb2 if i < 2 else xb3)
                rope_sg(sg)

            # ---------- phase B: batch-1 v-projection ----------
            for sg in range(4, 8):
                v_sg(sg, xb2 if sg < 6 else xb3)

            # ---------- phase D: batch-1 attention ----------
            for qs in range(4):
                attn_pass(1, qs)

            # ---------- phase E: AllToAll + output projection ----------
            nc.gpsimd.collective_compute(
                "AllToAll", mybir.AluOpType.bypass,
                replica_groups=[list(range(NC))],
                ins=[a2a_in.ap().opt()],
                outs=[a2a_out.ap().opt()])

            _cmB.__exit__(None, None, None)
            _cmA.__exit__(None, None, None)
            _cmO = tc.tile_pool(name="psO", bufs=1, space="PSUM")
            psO = _cmO.__enter__()

            # Keep the PE array (HAM) warm across the AllToAll wait: a chain
            # of full-array matmuls anchored on the last ctx tile (via the
            # dumsrc copy) so they cannot run before attention finishes.
            # Result is consumed by a dead-store DMA so DCE keeps the chain.
            dumsrc = pp.tile([128, 512], bf16, tag="dumsrc")
            nc.gpsimd.memset(dumsrc[:, :], 0.0)
            nc.vector.tensor_copy(
                dumsrc[0:64, :], ctxh[1][:, SB - 512:SB])
            dum = psO.tile([128, 512], f32, tag="dum", bufs=1)
            for i in range(N_DUMMY):
                nc.tensor.matmul(
                    dum[:, :], lhsT=wo_sb[:, 0:128], rhs=dumsrc[:, :],
                    start=True, stop=True)
            dumr = ep.tile([128, 512], f32, tag="dumr")
            nc.vector.tensor_copy(dumr[:, :], dum[:, :])
            dead = nc.dram_tensor("dead", [128, 512], f32)
            nc.sync.dma_start(out=dead[:, :], in_=dumr[:, :])
            # Load all 8 received o-chunks (split across two queues), then
            # run the accumulation ot-outer so each out-tile finishes early
            # and its bias-add + store overlap the remaining matmuls.
            cxs = []
            for c in range(NHC):
                cx = pp.tile([128, RB], bf16, tag=f"cxb{c}", name=f"cxb{c}")
                eng = nc.sync if c % 2 == 0 else nc.gpsimd
                eng.dma_start(out=cx[:, :], in_=a2a_out[c, :, :])
                cxs.append(cx)
            for ot in range(8):
                ops = psO.tile([128, 512], f32, tag="ops", bufs=4)
                for c in range(NHC):
                    nc.tensor.matmul(
                        ops[:, :],
                        lhsT=wo_sb[:, 1024 * c + 128 * ot:
                                   1024 * c + 128 * (ot + 1)],
                        rhs=cxs[c][:, :],
                        start=(c == 0), stop=(c == NHC - 1))
                osb = ep.tile([128, RB], bf16, tag="osb", bufs=3)
                nc.scalar.activation(
                    osb[:, :], ops[:, :], AF.Identity,
                    bias=bo_sb[:, ot:ot + 1], scale=1.0)
                eng = nc.sync if ot % 2 == 0 else nc.gpsimd
                eng.dma_start(
                    out=out_ext[128 * ot:128 * (ot + 1), :], in_=osb[:, :])
            _cmO.__exit__(None, None, None)

    nc.finalize()
    return nc


def _host_tables():
    inv = 1.0 / (ROPE_BASE ** (np.arange(0, D, 2, dtype=np.float64) / D))
    pos = np.arange(S, dtype=np.float64)
    freqs = np.outer(pos, inv)                      # [S, 32]
    emb = np.concatenate([freqs, freqs], axis=-1)   # [S, 64]
    cosT = np.cos(emb).T.astype(np.float32)         # [64, S]
    sinT = np.sin(emb).T.astype(np.float32)
    sinS = np.concatenate([-sinT[:32], sinT[32:]], axis=0)
    cos2 = np.ascontiguousarray(np.tile(cosT, (2, 2)))   # [128, 2S]
    sin2 = np.ascontiguousarray(np.tile(sinS, (2, 2)))
    return cos2, sin2


def _pack_wqkv(Wq, Wk, Wv, sl, bf):
    # [128, 3*1024]: col t*1024 + c*128 + j holds W_t.T[128c+p, j] for
    # partition p (pre-transposed row chunks of each projection).
    out = np.empty((128, 3 * 1024), dtype=np.float32)
    for t, W in enumerate((Wq, Wk, Wv)):
        wt = W[sl, :].T.reshape(8, 128, 128)          # [c, p, j]
        out[:, 1024 * t:1024 * (t + 1)] = (
            wt.transpose(1, 0, 2).reshape(128, 1024))
    return np.ascontiguousarray(out).astype(bf)


def kernel(**inputs):
    import ml_dtypes
    from concourse.bass_utils import run_bass_kernel_spmd

    global _cached, _last_in_maps
    if _cached is None:
        _cached = _build_nc()
    nc = _cached

    bf = ml_dtypes.bfloat16
    hs = np.asarray(inputs["hidden_states"], dtype=np.float32)
    Wq = np.asarray(inputs["Wq"], dtype=np.float32)
    bq = np.asarray(inputs["bq"], dtype=np.float32)
    Wk = np.asarray(inputs["Wk"], dtype=np.float32)
    bk = np.asarray(inputs["bk"], dtype=np.float32)
    Wv = np.asarray(inputs["Wv"], dtype=np.float32)
    bv = np.asarray(inputs["bv"], dtype=np.float32)
    Wo = np.asarray(inputs["Wo"], dtype=np.float32)
    bo = np.asarray(inputs["bo"], dtype=np.float32)

    cos2, sin2 = _host_tables()
    cs = np.ascontiguousarray(
        np.concatenate([cos2, sin2], axis=1)).astype(bf)   # [128, 2SB]
    bo2 = bo + bv @ Wo.T                                 # fold v-bias exactly
    bo2m = np.ascontiguousarray(bo2.reshape(8, 128).T)   # [128, 8]
    xTfull = np.ascontiguousarray(
        np.concatenate([hs[0].T, hs[1].T], axis=1)).astype(bf)  # [1024, 4096]
    # woL[p, 1024c + m] = Wo.T[128c+p, m]
    woL = np.ascontiguousarray(
        Wo.T.reshape(8, 128, 1024).transpose(1, 0, 2).reshape(128, 8192)
    ).astype(bf)
    # 32-row half-rotation permutation: dest row m reads source row perm(m)
    pidx = np.arange(128)
    pm = np.where(pidx % 64 < 32, pidx + 32, pidx - 32)
    permM = np.zeros((128, 128), dtype=np.float32)
    permM[pm, pidx] = 1.0                                # [k, m]: k==perm(m)
    permM = permM.astype(bf)

    in_maps = []
    for c in range(NC):
        sl = slice(OSL * c, OSL * (c + 1))
        bqk = np.stack([bq[sl] * 0.125, bk[sl]], axis=1)  # [128, 2]
        in_maps.append({
            "xT": xTfull,
            "wqkv": _pack_wqkv(Wq, Wk, Wv, sl, bf),
            "woL": woL,
            "bqk": np.ascontiguousarray(bqk.astype(np.float32)),
            "bo2": bo2m,
            "cs": cs,
            "perm": permM,
        })

    _last_in_maps = in_maps
    res = run_bass_kernel_spmd(nc, in_maps, core_ids=list(range(NC)))
    out = np.empty((2, S, HID), dtype=np.float32)
    for c in range(NC):
        b, g = divmod(c, 4)
        out[b, RB * g:RB * (g + 1), :] = res.results[c]["out"].T.astype(np.float32)
    return out
